# revision 10
# baseline (speedup 1.0000x reference)
"""MoE routing kernel for Trainium2 (Bass/Tile), 8-core data-parallel.

Problem: out = einsum('be,beo->bo', softmax(x@Wg+bg, axis=1),
                      einsum('bd,edo->beo', x, We) + be)
with B=8192, D=1024, O=1024, E=8 (all experts dense, softmax-weighted).

Strategy: shard the batch across 8 NeuronCores (1024 rows each). Each core:
  - gates computed TRANSPOSED: stationary Wg chunk [128,8] (M=8), moving xT
    halves -> psum gT [8, 512] x2 packed in ONE bank (col-groups 0/32);
    16 N=512 matmuls instead of 64 N=8 ones, k-ordered so they consume xT
    chunks as the DMAs land,
  - softmax: ACT Exp with per-partition bias bg[8,1] -> gT_exp [8,1024] bf16;
    thin PE transposes ([8,128] -> [128,8], ~110ns) + DVE tree-sum + recip
    give per-m normalized gates g_sb [128,8] and rden [128,1],
  - early phase: expert-0 n0 psums for m0..6 accumulate k-ordered (5 eps +
    2 bps banks) interleaved with the gate matmuls, so real PE work starts
    as soon as xT chunk0 + We[0] chunk0 arrive (~9.5us) instead of waiting
    for all of xT,
  - main loop e-outer / m-mid / k-inner with n0/n1 paired per (m,k) for
    stationary reuse; combine per (m,n) via fused DVE
    scalar_tensor_tensor acc = psum_e*g[:,e] + acc,
  - bias term: pb = gT_exp(unnormalized).T @ be per (m,n); folded with
    acc = pb*rden + acc (normalization by rden happens in the fold),
  - output DMAs alternate scalar/sync queues, issued per (m,n) right after
    the final fold.
Inputs are cast to bf16 host-side; x pre-transposed to [P, KC*BS]; We
re-laid out to [E, P, (k,n)-major] so per-chunk DMAs are contiguous.
"""
from contextlib import ExitStack

import numpy as np
import ml_dtypes

import concourse.tile as tile
import concourse.mybir as mybir
from concourse import bacc
from concourse.bass_utils import run_bass_kernel_spmd
from concourse.masks import make_identity

B, D, O, E = 8192, 1024, 1024, 8
NCORES = 8
BS = B // NCORES          # batch rows per core
P = 128                   # partition dim
NT = 512                  # matmul moving free-dim / PSUM bank width (fp32)
KC = D // P               # contraction chunks (8)
MC = BS // P              # batch-row chunks per core (8)
NCH = O // NT             # output column chunks (2)

F32 = mybir.dt.float32
BF16 = mybir.dt.bfloat16
MULT = mybir.AluOpType.mult
ADD = mybir.AluOpType.add

N_WARM = 24               # PE warm-up matmuls (cover preamble+first DMA wait)
M_EARLY = 7               # m-tiles of expert-0/n0 accumulated k-ordered early


def _emit(nc, tc, xT, We, Wg, bg, be, out):
    ctx = ExitStack()
    with ctx:
        const = ctx.enter_context(tc.tile_pool(name="const", bufs=1))
        xp = ctx.enter_context(tc.tile_pool(name="xp", bufs=1))
        wp = ctx.enter_context(tc.tile_pool(name="wp", bufs=1))
        gp = ctx.enter_context(tc.tile_pool(name="gp", bufs=1))
        accp = ctx.enter_context(tc.tile_pool(name="accp", bufs=1))
        small = ctx.enter_context(tc.tile_pool(name="small", bufs=2))
        gps = ctx.enter_context(tc.tile_pool(name="gps", bufs=1, space="PSUM"))
        bps = ctx.enter_context(tc.tile_pool(name="bps", bufs=2, space="PSUM"))
        eps = ctx.enter_context(tc.tile_pool(name="eps", bufs=5, space="PSUM"))

        # ---- DMA issue plan ----
        # Engines block IN ORDER on their DMA queue's flow-control
        # semaphores (4 outstanding per queue), so an engine that has
        # critical compute later must issue only a few DMAs:
        #   scalar: bg, wg, xt k0-1, we0 k0-1, xt k4-5, we0 k4-5
        #           -> then free for the ACT exps / copies / out-n0 DMAs
        #   sync:   xt k2-3, we0 k2-3, xt k6-7, we0 k6-7, we1..7 top halves
        #   gpsimd: be, we1..7 bottom halves (gpsimd has no other work)
        # xT / We[0] move as [128, 4KB] k-pair lines (2KB-line DMAs are
        # packet-rate-bound and crawl at ~250 B/ns aggregate).
        bg_sb = const.tile([E, 1], F32, name="bg_sb")
        nc.scalar.dma_start(bg_sb[:], bg)
        wg_all = const.tile([P, KC * E], BF16, name="wg_all")
        nc.scalar.dma_start(
            wg_all[:].rearrange("p (k e) -> p k e", k=KC),
            Wg.rearrange("(k p) e -> p k e", p=P))

        xt_all = xp.tile([P, KC * BS], BF16, name="xt_all")
        we_all = [wp.tile([P, KC * O], BF16, name=f"we{e}", tag=f"we{e}")
                  for e in range(E)]

        def xt(k, ms):
            return xt_all[:, k * BS + ms.start:k * BS + ms.stop]

        def wg(k):
            return wg_all[:, k * E:(k + 1) * E]

        def we(e, k, n):
            c = (k * NCH + n) * NT
            return we_all[e][:, c:c + NT]

        def dma_xt2(eng, k):          # chunks k, k+1
            eng.dma_start(xt_all[:, k * BS:(k + 2) * BS],
                          xT[:, k * BS:(k + 2) * BS])

        def dma_we2(eng, e, k):       # chunks k, k+1 (both n-halves)
            c = k * NCH * NT
            eng.dma_start(we_all[e][:, c:c + 2 * NCH * NT],
                          We[e, :, c:c + 2 * NCH * NT])

        dma_xt2(nc.scalar, 0)
        dma_xt2(nc.sync, 2)
        dma_we2(nc.scalar, 0, 0)
        dma_we2(nc.sync, 0, 2)
        dma_xt2(nc.scalar, 4)
        dma_xt2(nc.sync, 6)
        dma_we2(nc.scalar, 0, 4)
        dma_we2(nc.sync, 0, 6)

        ident8 = const.tile([E, E], BF16, name="ident8")
        make_identity(nc, ident8[:])

        be_sb = const.tile([E, O], BF16, name="be_sb")
        nc.gpsimd.dma_start(be_sb[:], be)

        # bulk expert weights: bottom halves on gpsimd, top halves on sync
        HALF = KC * O // 2
        for e in range(1, E):
            nc.gpsimd.dma_start(we_all[e][:, :HALF], We[e, :, :HALF])
        for e in range(1, E):
            nc.sync.dma_start(we_all[e][:, HALF:], We[e, :, HALF:])

        # ---- PE warm-up ----
        # HAM keeps the PE clock-gated at 1.2 GHz until ~3.4us of sustained
        # matmul activity; the first ~8us is also DMA-wait (framework
        # preamble + first chunk transfers). Burn warm-up matmuls there.
        warm_sb = const.tile([P, NT], BF16, name="warm_sb")
        nc.vector.memset(warm_sb[:], 0.0)

        def warmup(n):
            for _ in range(n):
                pwu = bps.tile([P, NT], F32, name="pwu", tag="pb")
                nc.tensor.matmul(pwu[:], warm_sb[:, :P], warm_sb[:],
                                 start=True, stop=True)

        warmup(N_WARM)

        # ---- chunk-paced phase: gates (transposed) + expert-0/n0 ----
        # gate psums: two [8, 512] tiles packed into one PSUM bank at
        # base partitions 0 and 32 (col-groups run concurrently on PE)
        g4 = gps.tile([40, NT], F32, name="g4", tag="gq")
        gpsum = [g4[0:E, :], g4[32:32 + E, :]]

        # early expert-0 n0 psums: m0..4 from the shared eps ring (bufs=5),
        # m5..6 from the bps ring (bufs=2) — ring FIFO order matches the
        # seed order below, so the main loop's allocations naturally wait
        # on the seeds
        early_pe = []
        for m in range(M_EARLY):
            pool, tag = (eps, "pe") if m < 5 else (bps, "pb")
            t = pool.tile([P, NT], F32, name=f"pe_e{m}", tag=tag)
            early_pe.append(t)

        # gates first (paced only by the xT DMAs) so the softmax pipeline
        # starts as early as possible
        for k in range(KC):
            for h in range(2):
                hs = slice(h * NT, (h + 1) * NT)
                nc.tensor.matmul(gpsum[h], wg(k), xt(k, hs),
                                 start=(k == 0), stop=(k == KC - 1))

        def early_block(ks):
            for k in ks:
                for m in range(M_EARLY):
                    ms = slice(m * P, (m + 1) * P)
                    nc.tensor.matmul(early_pe[m][:], xt(k, ms), we(0, k, 0),
                                     start=(k == 0), stop=(k == KC - 1))

        early_block([0, 1])

        # softmax numerator: exp(logits + bg), no max-subtraction (logits
        # are bounded, |logit| < ~3 for this input distribution)
        gT_exp = gp.tile([E, BS], BF16, name="gT_exp")
        for h in range(2):
            hs = slice(h * NT, (h + 1) * NT)
            nc.scalar.activation(gT_exp[:, hs], gpsum[h],
                                 mybir.ActivationFunctionType.Exp,
                                 bias=bg_sb[:], scale=1.0)

        # transposes: [8, 128] -> [128, 8], all packed into one PSUM bank
        # (same ring slot as the gate psums — waits for the ACT reads);
        # then per-m ACT copy + accum + recip + normalize
        tp = gps.tile([P, E * MC], BF16, name="tp", tag="gq")
        accs = [[accp.tile([P, NT], F32, name=f"acc{m}_{n}", tag=f"acc{m}_{n}")
                 for n in range(NCH)] for m in range(MC)]
        g_sb, rden_sb = [], []
        for m in range(MC):
            ms = slice(m * P, (m + 1) * P)
            es = slice(m * E, (m + 1) * E)
            nc.tensor.transpose(tp[:, es], gT_exp[:, ms], ident8[:])
        early_block([2, 3, 4, 5, 6, 7])
        for m in range(MC):
            # per-m: ACT copy psum->sbuf with accumulate giving the softmax
            # denominator, then recip + normalize; the expert-0/n0 seed
            # follows immediately so each psum ring slot frees early
            es = slice(m * E, (m + 1) * E)
            ge = gp.tile([P, E], F32, name=f"ge{m}", tag=f"ge{m}")
            den = small.tile([P, 1], F32, name="den", tag="den")
            nc.scalar.activation(ge[:], tp[:, es],
                                 mybir.ActivationFunctionType.Copy,
                                 accum_out=den[:])
            rden = gp.tile([P, 1], F32, name=f"rden{m}", tag=f"rden{m}")
            nc.vector.reciprocal(rden[:], den[:])
            g = gp.tile([P, E], F32, name=f"g{m}", tag=f"g{m}")
            nc.vector.tensor_scalar_mul(g[:], ge[:], rden[:])
            g_sb.append(g)
            rden_sb.append(rden)
            if m < M_EARLY:
                nc.vector.tensor_scalar_mul(accs[m][0][:], early_pe[m][:],
                                            g[:, 0:1])

        def expert_group(e, m, ns):
            """Emit matmuls for expert e, m-tile m, n-chunks ns (paired
            k-inner for stationary locality); returns psums."""
            ms = slice(m * P, (m + 1) * P)
            pes = {}
            for n in ns:
                pes[n] = eps.tile([P, NT], F32, name=f"pe{n}", tag="pe")
            for k in range(KC):
                for n in ns:
                    nc.tensor.matmul(pes[n][:], xt(k, ms), we(e, k, n),
                                     start=(k == 0), stop=(k == KC - 1))
            return pes

        def combine(e, m, n, pe):
            if e == 0:
                nc.vector.tensor_scalar_mul(accs[m][n][:], pe[:],
                                            g_sb[m][:, 0:1])
            else:
                nc.vector.scalar_tensor_tensor(
                    accs[m][n][:], pe[:], g_sb[m][:, e:e + 1],
                    accs[m][n][:], MULT, ADD)

        # ---- expert 0: finish n1 for early m-tiles, both n for the rest
        for m in range(MC):
            ns = [1] if m < M_EARLY else [0, 1]
            pes = expert_group(0, m, ns)
            for n in ns:
                combine(0, m, n, pes[n])

        # ---- experts 1..7 ----
        for e in range(1, E):
            for m in range(MC):
                pes = expert_group(e, m, [0, 1])
                if e == 1:
                    # bias term: pb = gT_exp(unnorm).T @ be; the fold
                    # multiplies by rden, completing the normalization
                    ms = slice(m * P, (m + 1) * P)
                    for n in range(NCH):
                        pb = bps.tile([P, NT], F32, name="pb", tag="pb")
                        nc.tensor.matmul(pb[:], gT_exp[:, ms],
                                         be_sb[:, n * NT:(n + 1) * NT],
                                         start=True, stop=True)
                        nc.vector.scalar_tensor_tensor(
                            accs[m][n][:], pb[:], rden_sb[m][:],
                            accs[m][n][:], MULT, ADD)
                for n in range(NCH):
                    combine(e, m, n, pes[n])
                if e == E - 1:
                    ms = slice(m * P, (m + 1) * P)
                    for n in range(NCH):
                        eng = nc.scalar if n == 0 else nc.sync
                        eng.dma_start(out[ms, n * NT:(n + 1) * NT],
                                      accs[m][n][:])


_NC_CACHE = {}


def _build():
    if "nc" in _NC_CACHE:
        return _NC_CACHE["nc"]
    nc = bacc.Bacc("TRN2", target_bir_lowering=False, debug=False,
                   num_devices=NCORES)
    xT = nc.dram_tensor("xT", [P, KC * BS], BF16, kind="ExternalInput").ap()
    We_t = nc.dram_tensor("We", [E, P, KC * O], BF16,
                          kind="ExternalInput").ap()
    Wg_t = nc.dram_tensor("Wg", [D, E], BF16, kind="ExternalInput").ap()
    bg_t = nc.dram_tensor("bg", [E, 1], F32, kind="ExternalInput").ap()
    be_t = nc.dram_tensor("be", [E, O], BF16, kind="ExternalInput").ap()
    out = nc.dram_tensor("out", [BS, O], F32, kind="ExternalOutput").ap()
    with tile.TileContext(nc) as tc:
        _emit(nc, tc, xT, We_t, Wg_t, bg_t, be_t, out)
    nc.compile()
    _NC_CACHE["nc"] = nc
    return nc


def _in_maps(x, Wg, bg, We, be):
    bf = ml_dtypes.bfloat16
    x = np.asarray(x, dtype=np.float32)
    # We re-laid out so that for each k-chunk the (n0, n1) slices are
    # contiguous: We_r[e, p, (k*2+n)*512 + j] = We[e, k*128+p, n*512+j]
    We_bf = np.ascontiguousarray(
        np.asarray(We, dtype=np.float32).astype(bf)
        .reshape(E, KC, P, NCH, NT).transpose(0, 2, 1, 3, 4)
        .reshape(E, P, KC * O))
    Wg_bf = np.asarray(Wg, dtype=np.float32).astype(bf)
    be_bf = np.asarray(be, dtype=np.float32).astype(bf)
    bg32 = np.asarray(bg, dtype=np.float32).reshape(E, 1)
    maps = []
    for c in range(NCORES):
        # xT_r[p, k*BS + b] = x[c*BS + b, k*P + p]
        xs = x[c * BS:(c + 1) * BS].astype(bf)        # [BS, D]
        xT = np.ascontiguousarray(
            xs.reshape(BS, KC, P).transpose(2, 1, 0).reshape(P, KC * BS))
        maps.append({"xT": xT, "We": We_bf, "Wg": Wg_bf,
                     "bg": bg32, "be": be_bf})
    return maps


def run(x, Wg, bg, We, be, **spmd_kwargs):
    nc = _build()
    maps = _in_maps(x, Wg, bg, We, be)
    res = run_bass_kernel_spmd(nc, maps, core_ids=list(range(NCORES)),
                               **spmd_kwargs)
    out = np.concatenate([res.results[c]["out"] for c in range(NCORES)],
                         axis=0)
    return out, res


def kernel(x, Wg, bg, We, be):
    out, _ = run(x, Wg, bg, We, be)
    return out


# revision 12
# speedup vs baseline: 1.1088x; 1.1088x over previous
"""MoE routing kernel for Trainium2 (Bass/Tile), 8-core data-parallel.

Problem: out = einsum('be,beo->bo', softmax(x@Wg+bg, axis=1),
                      einsum('bd,edo->beo', x, We) + be)
with B=8192, D=1024, O=1024, E=8 (all experts dense, softmax-weighted).

Strategy: shard the batch across 8 NeuronCores (1024 rows each). Each core:
  - gates computed TRANSPOSED: stationary Wg chunk [128,8] (M=8), moving xT
    halves -> psum gT [8, 512] x2 packed in ONE bank (col-groups 0/32);
    16 N=512 matmuls instead of 64 N=8 ones, k-ordered so they consume xT
    chunks as the DMAs land,
  - softmax: ACT Exp with per-partition bias bg[8,1] -> gT_exp [8,1024] bf16;
    thin PE transposes ([8,128] -> [128,8], ~110ns) + DVE tree-sum + recip
    give per-m normalized gates g_sb [128,8] and rden [128,1],
  - early phase: expert-0 n0 psums for m0..6 accumulate k-ordered (5 eps +
    2 bps banks) interleaved with the gate matmuls, so real PE work starts
    as soon as xT chunk0 + We[0] chunk0 arrive (~9.5us) instead of waiting
    for all of xT,
  - main loop e-outer / m-mid / k-inner with n0/n1 paired per (m,k) for
    stationary reuse; combine per (m,n) via fused DVE
    scalar_tensor_tensor acc = psum_e*g[:,e] + acc,
  - bias term: pb = gT_exp(unnormalized).T @ be per (m,n); folded with
    acc = pb*rden + acc (normalization by rden happens in the fold),
  - output DMAs alternate scalar/sync queues, issued per (m,n) right after
    the final fold.
Inputs are cast to bf16 host-side; x pre-transposed to [P, KC*BS]; We
re-laid out to [E, P, (k,n)-major] so per-chunk DMAs are contiguous.
"""
from contextlib import ExitStack

import numpy as np
import ml_dtypes

import concourse.tile as tile
import concourse.mybir as mybir
from concourse import bacc
from concourse.bass_utils import run_bass_kernel_spmd
from concourse.masks import make_identity

B, D, O, E = 8192, 1024, 1024, 8
NCORES = 8
BS = B // NCORES          # batch rows per core
P = 128                   # partition dim
NT = 512                  # matmul moving free-dim / PSUM bank width (fp32)
KC = D // P               # contraction chunks (8)
MC = BS // P              # batch-row chunks per core (8)
NCH = O // NT             # output column chunks (2)

F32 = mybir.dt.float32
BF16 = mybir.dt.bfloat16
MULT = mybir.AluOpType.mult
ADD = mybir.AluOpType.add

N_WARM = 24               # PE warm-up matmuls (cover preamble+first DMA wait)
M_EARLY = 7               # m-tiles of expert-0/n0 accumulated k-ordered early


def _emit(nc, tc, xT, We, Wg, bg, be, out):
    ctx = ExitStack()
    with ctx:
        const = ctx.enter_context(tc.tile_pool(name="const", bufs=1))
        xp = ctx.enter_context(tc.tile_pool(name="xp", bufs=1))
        wp = ctx.enter_context(tc.tile_pool(name="wp", bufs=1))
        gp = ctx.enter_context(tc.tile_pool(name="gp", bufs=1))
        accp = ctx.enter_context(tc.tile_pool(name="accp", bufs=1))
        small = ctx.enter_context(tc.tile_pool(name="small", bufs=2))
        gps = ctx.enter_context(tc.tile_pool(name="gps", bufs=1, space="PSUM"))
        bps = ctx.enter_context(tc.tile_pool(name="bps", bufs=2, space="PSUM"))
        eps = ctx.enter_context(tc.tile_pool(name="eps", bufs=5, space="PSUM"))

        # ---- DMA issue plan ----
        # Per-core DMA bandwidth is a SHARED ~400 B/ns pool across all
        # queues, so the critical head data (xT + We[0], 4MB) must not
        # compete with the 14MB weight bulk: the bulk goes on the sync
        # queue BEHIND its critical items. Engines block IN ORDER on
        # their DMA queue's flow-control semaphores (4 outstanding), so
        # scalar (which must run the ACT softmax at ~16us) issues only 7:
        #   scalar: bg, wg, xt k0-1, we0 k0-1, xt k4-5, we0 k4-5, be
        #   sync:   xt k2-3, we0 k2-3, xt k6-7, we0 k6-7, then we1..7
        # xT / We[0] move as [128, 4KB] k-pair lines (2KB-line DMAs are
        # packet-rate-bound and crawl at ~250 B/ns aggregate).
        bg_sb = const.tile([E, 1], F32, name="bg_sb")
        nc.scalar.dma_start(bg_sb[:], bg)
        wg_all = const.tile([P, KC * E], BF16, name="wg_all")
        nc.scalar.dma_start(
            wg_all[:].rearrange("p (k e) -> p k e", k=KC),
            Wg.rearrange("(k p) e -> p k e", p=P))

        xt_all = xp.tile([P, KC * BS], BF16, name="xt_all")
        we_all = [wp.tile([P, KC * O], BF16, name=f"we{e}", tag=f"we{e}")
                  for e in range(E)]

        def xt(k, ms):
            return xt_all[:, k * BS + ms.start:k * BS + ms.stop]

        def wg(k):
            return wg_all[:, k * E:(k + 1) * E]

        def we(e, k, n):
            c = (k * NCH + n) * NT
            return we_all[e][:, c:c + NT]

        def dma_xt2(eng, k):          # chunks k, k+1
            eng.dma_start(xt_all[:, k * BS:(k + 2) * BS],
                          xT[:, k * BS:(k + 2) * BS])

        def dma_we2(eng, e, k):       # chunks k, k+1 (both n-halves)
            c = k * NCH * NT
            eng.dma_start(we_all[e][:, c:c + 2 * NCH * NT],
                          We[e, :, c:c + 2 * NCH * NT])

        dma_xt2(nc.scalar, 0)
        dma_xt2(nc.sync, 2)
        dma_we2(nc.scalar, 0, 0)
        dma_we2(nc.sync, 0, 2)
        dma_xt2(nc.scalar, 4)
        dma_xt2(nc.sync, 6)
        dma_we2(nc.scalar, 0, 4)
        dma_we2(nc.sync, 0, 6)

        ident8 = const.tile([E, E], BF16, name="ident8")
        make_identity(nc, ident8[:])

        be_sb = const.tile([E, O], BF16, name="be_sb")
        nc.scalar.dma_start(be_sb[:], be)

        # bulk expert weights: all on sync, behind its critical items
        HALF = KC * O // 2
        for e in range(1, E):
            for h in range(2):
                nc.sync.dma_start(we_all[e][:, h * HALF:(h + 1) * HALF],
                                  We[e, :, h * HALF:(h + 1) * HALF])

        # ---- PE warm-up ----
        # HAM keeps the PE clock-gated at 1.2 GHz until ~3.4us of sustained
        # matmul activity; the first ~8us is also DMA-wait (framework
        # preamble + first chunk transfers). Burn warm-up matmuls there.
        warm_sb = const.tile([P, NT], BF16, name="warm_sb")
        nc.vector.memset(warm_sb[:], 0.0)

        def warmup(n):
            for _ in range(n):
                pwu = bps.tile([P, NT], F32, name="pwu", tag="pb")
                nc.tensor.matmul(pwu[:], warm_sb[:, :P], warm_sb[:],
                                 start=True, stop=True)

        warmup(N_WARM)

        # ---- chunk-paced phase: gates (transposed) + expert-0/n0 ----
        # gate psums: two [8, 512] tiles packed into one PSUM bank at
        # base partitions 0 and 32 (col-groups run concurrently on PE)
        g4 = gps.tile([40, NT], F32, name="g4", tag="gq")
        gpsum = [g4[0:E, :], g4[32:32 + E, :]]

        # early expert-0 n0 psums: m0..4 from the shared eps ring (bufs=5),
        # m5..6 from the bps ring (bufs=2) — ring FIFO order matches the
        # seed order below, so the main loop's allocations naturally wait
        # on the seeds
        early_pe = []
        for m in range(M_EARLY):
            pool, tag = (eps, "pe") if m < 5 else (bps, "pb")
            t = pool.tile([P, NT], F32, name=f"pe_e{m}", tag=tag)
            early_pe.append(t)

        # gates first (paced only by the xT DMAs) so the softmax pipeline
        # starts as early as possible
        for k in range(KC):
            for h in range(2):
                hs = slice(h * NT, (h + 1) * NT)
                nc.tensor.matmul(gpsum[h], wg(k), xt(k, hs),
                                 start=(k == 0), stop=(k == KC - 1))

        def early_block(ks):
            for k in ks:
                for m in range(M_EARLY):
                    ms = slice(m * P, (m + 1) * P)
                    nc.tensor.matmul(early_pe[m][:], xt(k, ms), we(0, k, 0),
                                     start=(k == 0), stop=(k == KC - 1))

        early_block([0, 1])

        # softmax numerator: exp(logits + bg), no max-subtraction (logits
        # are bounded, |logit| < ~3 for this input distribution)
        gT_exp = gp.tile([E, BS], BF16, name="gT_exp")
        for h in range(2):
            hs = slice(h * NT, (h + 1) * NT)
            nc.scalar.activation(gT_exp[:, hs], gpsum[h],
                                 mybir.ActivationFunctionType.Exp,
                                 bias=bg_sb[:], scale=1.0)

        # transposes: [8, 128] -> [128, 8], all packed into one PSUM bank
        # (same ring slot as the gate psums — waits for the ACT reads);
        # then per-m ACT copy + accum + recip + normalize
        tp = gps.tile([P, E * MC], BF16, name="tp", tag="gq")
        accs = [[accp.tile([P, NT], F32, name=f"acc{m}_{n}", tag=f"acc{m}_{n}")
                 for n in range(NCH)] for m in range(MC)]
        g_sb, rden_sb = [], []
        for m in range(MC):
            ms = slice(m * P, (m + 1) * P)
            es = slice(m * E, (m + 1) * E)
            nc.tensor.transpose(tp[:, es], gT_exp[:, ms], ident8[:])
        early_block([2, 3, 4, 5, 6, 7])
        for m in range(MC):
            # per-m: ACT copy psum->sbuf with accumulate giving the softmax
            # denominator, then recip + normalize; the expert-0/n0 seed
            # follows immediately so each psum ring slot frees early
            es = slice(m * E, (m + 1) * E)
            ge = gp.tile([P, E], F32, name=f"ge{m}", tag=f"ge{m}")
            den = small.tile([P, 1], F32, name="den", tag="den")
            nc.scalar.activation(ge[:], tp[:, es],
                                 mybir.ActivationFunctionType.Copy,
                                 accum_out=den[:])
            rden = gp.tile([P, 1], F32, name=f"rden{m}", tag=f"rden{m}")
            nc.vector.reciprocal(rden[:], den[:])
            g = gp.tile([P, E], F32, name=f"g{m}", tag=f"g{m}")
            nc.vector.tensor_scalar_mul(g[:], ge[:], rden[:])
            g_sb.append(g)
            rden_sb.append(rden)
            if m < M_EARLY:
                nc.vector.tensor_scalar_mul(accs[m][0][:], early_pe[m][:],
                                            g[:, 0:1])

        def expert_group(e, m, ns):
            """Emit matmuls for expert e, m-tile m, n-chunks ns (paired
            k-inner for stationary locality); returns psums."""
            ms = slice(m * P, (m + 1) * P)
            pes = {}
            for n in ns:
                pes[n] = eps.tile([P, NT], F32, name=f"pe{n}", tag="pe")
            for k in range(KC):
                for n in ns:
                    nc.tensor.matmul(pes[n][:], xt(k, ms), we(e, k, n),
                                     start=(k == 0), stop=(k == KC - 1))
            return pes

        def combine(e, m, n, pe):
            if e == 0:
                nc.vector.tensor_scalar_mul(accs[m][n][:], pe[:],
                                            g_sb[m][:, 0:1])
            else:
                nc.vector.scalar_tensor_tensor(
                    accs[m][n][:], pe[:], g_sb[m][:, e:e + 1],
                    accs[m][n][:], MULT, ADD)

        # ---- expert 0: finish n1 for early m-tiles, both n for the rest
        for m in range(MC):
            ns = [1] if m < M_EARLY else [0, 1]
            pes = expert_group(0, m, ns)
            for n in ns:
                combine(0, m, n, pes[n])

        # ---- experts 1..7 ----
        for e in range(1, E):
            for m in range(MC):
                pes = expert_group(e, m, [0, 1])
                if e == 1:
                    # bias term: pb = gT_exp(unnorm).T @ be; the fold
                    # multiplies by rden, completing the normalization
                    ms = slice(m * P, (m + 1) * P)
                    for n in range(NCH):
                        pb = bps.tile([P, NT], F32, name="pb", tag="pb")
                        nc.tensor.matmul(pb[:], gT_exp[:, ms],
                                         be_sb[:, n * NT:(n + 1) * NT],
                                         start=True, stop=True)
                        nc.vector.scalar_tensor_tensor(
                            accs[m][n][:], pb[:], rden_sb[m][:],
                            accs[m][n][:], MULT, ADD)
                for n in range(NCH):
                    combine(e, m, n, pes[n])
                if e == E - 1:
                    ms = slice(m * P, (m + 1) * P)
                    for n in range(NCH):
                        eng = nc.scalar if n == 0 else nc.sync
                        eng.dma_start(out[ms, n * NT:(n + 1) * NT],
                                      accs[m][n][:])


_NC_CACHE = {}


def _build():
    if "nc" in _NC_CACHE:
        return _NC_CACHE["nc"]
    nc = bacc.Bacc("TRN2", target_bir_lowering=False, debug=False,
                   num_devices=NCORES)
    xT = nc.dram_tensor("xT", [P, KC * BS], BF16, kind="ExternalInput").ap()
    We_t = nc.dram_tensor("We", [E, P, KC * O], BF16,
                          kind="ExternalInput").ap()
    Wg_t = nc.dram_tensor("Wg", [D, E], BF16, kind="ExternalInput").ap()
    bg_t = nc.dram_tensor("bg", [E, 1], F32, kind="ExternalInput").ap()
    be_t = nc.dram_tensor("be", [E, O], BF16, kind="ExternalInput").ap()
    out = nc.dram_tensor("out", [BS, O], F32, kind="ExternalOutput").ap()
    with tile.TileContext(nc) as tc:
        _emit(nc, tc, xT, We_t, Wg_t, bg_t, be_t, out)
    nc.compile()
    _NC_CACHE["nc"] = nc
    return nc


def _in_maps(x, Wg, bg, We, be):
    bf = ml_dtypes.bfloat16
    x = np.asarray(x, dtype=np.float32)
    # We re-laid out so that for each k-chunk the (n0, n1) slices are
    # contiguous: We_r[e, p, (k*2+n)*512 + j] = We[e, k*128+p, n*512+j]
    We_bf = np.ascontiguousarray(
        np.asarray(We, dtype=np.float32).astype(bf)
        .reshape(E, KC, P, NCH, NT).transpose(0, 2, 1, 3, 4)
        .reshape(E, P, KC * O))
    Wg_bf = np.asarray(Wg, dtype=np.float32).astype(bf)
    be_bf = np.asarray(be, dtype=np.float32).astype(bf)
    bg32 = np.asarray(bg, dtype=np.float32).reshape(E, 1)
    maps = []
    for c in range(NCORES):
        # xT_r[p, k*BS + b] = x[c*BS + b, k*P + p]
        xs = x[c * BS:(c + 1) * BS].astype(bf)        # [BS, D]
        xT = np.ascontiguousarray(
            xs.reshape(BS, KC, P).transpose(2, 1, 0).reshape(P, KC * BS))
        maps.append({"xT": xT, "We": We_bf, "Wg": Wg_bf,
                     "bg": bg32, "be": be_bf})
    return maps


def run(x, Wg, bg, We, be, **spmd_kwargs):
    nc = _build()
    maps = _in_maps(x, Wg, bg, We, be)
    res = run_bass_kernel_spmd(nc, maps, core_ids=list(range(NCORES)),
                               **spmd_kwargs)
    out = np.concatenate([res.results[c]["out"] for c in range(NCORES)],
                         axis=0)
    return out, res


def kernel(x, Wg, bg, We, be):
    out, _ = run(x, Wg, bg, We, be)
    return out


# revision 21
# speedup vs baseline: 1.2022x; 1.0842x over previous
"""MoE routing kernel for Trainium2 (Bass/Tile), 8-core data-parallel.

Problem: out = einsum('be,beo->bo', softmax(x@Wg+bg, axis=1),
                      einsum('bd,edo->beo', x, We) + be)
with B=8192, D=1024, O=1024, E=8 (all experts dense, softmax-weighted).

Strategy: shard the batch across 8 NeuronCores (1024 rows each). Each core:
  - gates computed TRANSPOSED: stationary Wg chunk [128,8] (M=8), moving xT
    halves -> psum gT [8, 512] x2 packed in ONE bank (col-groups 0/32);
    16 N=512 matmuls instead of 64 N=8 ones, k-ordered so they consume xT
    chunks as the DMAs land,
  - softmax: ACT Exp with per-partition bias bg[8,1] -> gT_exp [8,1024] bf16;
    thin PE transposes ([8,128] -> [128,8], ~110ns) + DVE tree-sum + recip
    give per-m normalized gates g_sb [128,8] and rden [128,1],
  - early phase: expert-0 n0 psums for m0..6 accumulate k-ordered (5 eps +
    2 bps banks) interleaved with the gate matmuls, so real PE work starts
    as soon as xT chunk0 + We[0] chunk0 arrive (~9.5us) instead of waiting
    for all of xT,
  - main loop e-outer / m-mid / k-inner with n0/n1 paired per (m,k) for
    stationary reuse; combine per (m,n) via fused DVE
    scalar_tensor_tensor acc = psum_e*g[:,e] + acc,
  - bias term: pb = gT_exp(unnormalized).T @ be per (m,n); folded with
    acc = pb*rden + acc (normalization by rden happens in the fold),
  - output DMAs alternate scalar/sync queues, issued per (m,n) right after
    the final fold.
Inputs are cast to bf16 host-side; x pre-transposed to [P, KC*BS]; We
re-laid out to [E, P, (k,n)-major] so per-chunk DMAs are contiguous.
"""
from contextlib import ExitStack

import numpy as np
import ml_dtypes

import concourse.tile as tile
import concourse.mybir as mybir
from concourse import bacc
from concourse.bass_utils import run_bass_kernel_spmd
from concourse.masks import make_identity

B, D, O, E = 8192, 1024, 1024, 8
NCORES = 8
BS = B // NCORES          # batch rows per core
P = 128                   # partition dim
NT = 512                  # matmul moving free-dim / PSUM bank width (fp32)
KC = D // P               # contraction chunks (8)
MC = BS // P              # batch-row chunks per core (8)
NCH = O // NT             # output column chunks (2)

F32 = mybir.dt.float32
BF16 = mybir.dt.bfloat16
F8 = mybir.dt.float8e4
MULT = mybir.AluOpType.mult
ADD = mybir.AluOpType.add

N_WARM = 24               # PE warm-up matmuls (cover preamble+first DMA wait)
M_EARLY = 7               # m-tiles of expert-0/n0 accumulated k-ordered early
N_F8 = 2                  # trailing experts computed in fp8 DoubleRow (2x PE)
XS, WS = 16.0, 2048.0     # fp8 scales for x and We (keep values < 240)
ISCALE = 1.0 / (XS * WS)  # folded into the combine gates for fp8 experts


def _emit(nc, tc, xT, We, Wg, bg, be, xT8, We8, out):
    ctx = ExitStack()
    with ctx:
        const = ctx.enter_context(tc.tile_pool(name="const", bufs=1))
        xp = ctx.enter_context(tc.tile_pool(name="xp", bufs=1))
        wp = ctx.enter_context(tc.tile_pool(name="wp", bufs=1))
        gp = ctx.enter_context(tc.tile_pool(name="gp", bufs=1))
        accp = ctx.enter_context(tc.tile_pool(name="accp", bufs=1))
        small = ctx.enter_context(tc.tile_pool(name="small", bufs=2))
        gps = ctx.enter_context(tc.tile_pool(name="gps", bufs=1, space="PSUM"))
        bps = ctx.enter_context(tc.tile_pool(name="bps", bufs=2, space="PSUM"))
        eps = ctx.enter_context(tc.tile_pool(name="eps", bufs=5, space="PSUM"))

        # ---- DMA issue plan ----
        # Per-core DMA bandwidth is a SHARED ~400 B/ns pool across all
        # queues, so the critical head data (xT + We[0], 4MB) must not
        # compete with the 14MB weight bulk: the bulk goes on the sync
        # queue BEHIND its critical items. Engines block IN ORDER on
        # their DMA queue's flow-control semaphores (4 outstanding), so
        # scalar (which must run the ACT softmax at ~16us) issues only 7:
        #   scalar: bg, wg, xt k0-1, we0 k0-1, xt k4-5, we0 k4-5, be
        #   sync:   xt k2-3, we0 k2-3, xt k6-7, we0 k6-7, then we1..7
        # xT / We[0] move as [128, 4KB] k-pair lines (2KB-line DMAs are
        # packet-rate-bound and crawl at ~250 B/ns aggregate).
        bg_sb = const.tile([E, 1], F32, name="bg_sb")
        nc.scalar.dma_start(bg_sb[:], bg)
        wg_all = const.tile([P, KC * E], BF16, name="wg_all")
        nc.scalar.dma_start(
            wg_all[:].rearrange("p (k e) -> p k e", k=KC),
            Wg.rearrange("(k p) e -> p k e", p=P))

        xt_all = xp.tile([P, KC * BS], BF16, name="xt_all")
        we_all = [wp.tile([P, KC * O], BF16, name=f"we{e}", tag=f"we{e}")
                  for e in range(E - N_F8)]
        xt8_all = xp.tile([P, KC * BS], F8, name="xt8_all")
        we8_all = [wp.tile([P, KC * O], F8, name=f"we8_{i}", tag=f"we8_{i}")
                   for i in range(N_F8)]

        def xt(k, ms):
            return xt_all[:, k * BS + ms.start:k * BS + ms.stop]

        def wg(k):
            return wg_all[:, k * E:(k + 1) * E]

        def we(e, k, n):
            c = (k * NCH + n) * NT
            return we_all[e][:, c:c + NT]

        def dma_xt2(eng, k):          # chunks k, k+1
            eng.dma_start(xt_all[:, k * BS:(k + 2) * BS],
                          xT[:, k * BS:(k + 2) * BS])

        def dma_we2(eng, e, k):       # chunks k, k+1 (both n-halves)
            c = k * NCH * NT
            eng.dma_start(we_all[e][:, c:c + 2 * NCH * NT],
                          We[e, :, c:c + 2 * NCH * NT])

        dma_xt2(nc.scalar, 0)
        dma_xt2(nc.sync, 2)
        dma_we2(nc.scalar, 0, 0)
        dma_we2(nc.sync, 0, 2)
        dma_xt2(nc.scalar, 4)
        dma_xt2(nc.sync, 6)
        dma_we2(nc.scalar, 0, 4)
        dma_we2(nc.sync, 0, 6)

        ident8 = const.tile([E, E], BF16, name="ident8")
        make_identity(nc, ident8[:])

        be_sb = const.tile([E, O], BF16, name="be_sb")
        nc.scalar.dma_start(be_sb[:], be)

        # bulk expert weights: all on sync, behind its critical items;
        # the fp8 tail experts + fp8 x copy load last (needed latest)
        HALF = KC * O // 2
        for e in range(1, E - N_F8):
            for h in range(2):
                nc.sync.dma_start(we_all[e][:, h * HALF:(h + 1) * HALF],
                                  We[e, :, h * HALF:(h + 1) * HALF])
        nc.sync.dma_start(xt8_all[:], xT8)
        for i in range(N_F8):
            nc.sync.dma_start(we8_all[i][:], We8[i])

        # ---- PE warm-up ----
        # HAM keeps the PE clock-gated at 1.2 GHz until ~3.4us of sustained
        # matmul activity; the first ~8us is also DMA-wait (framework
        # preamble + first chunk transfers). Burn warm-up matmuls there.
        warm_sb = const.tile([P, NT], BF16, name="warm_sb")
        nc.vector.memset(warm_sb[:], 0.0)

        def warmup(n):
            for _ in range(n):
                pwu = bps.tile([P, NT], F32, name="pwu", tag="pb")
                nc.tensor.matmul(pwu[:], warm_sb[:, :P], warm_sb[:],
                                 start=True, stop=True)

        warmup(N_WARM)

        # ---- chunk-paced phase: gates (transposed) + expert-0/n0 ----
        # gate psums: two [8, 512] tiles packed into one PSUM bank at
        # base partitions 0 and 32 (col-groups run concurrently on PE)
        g4 = gps.tile([40, NT], F32, name="g4", tag="gq")
        gpsum = [g4[0:E, :], g4[32:32 + E, :]]

        # early expert-0 n0 psums: m0..4 from the shared eps ring (bufs=5),
        # m5..6 from the bps ring (bufs=2) — ring FIFO order matches the
        # seed order below, so the main loop's allocations naturally wait
        # on the seeds
        early_pe = []
        for m in range(M_EARLY):
            pool, tag = (eps, "pe") if m < 5 else (bps, "pb")
            t = pool.tile([P, NT], F32, name=f"pe_e{m}", tag=tag)
            early_pe.append(t)

        # gates first (paced only by the xT DMAs) so the softmax pipeline
        # starts as early as possible
        for k in range(KC):
            for h in range(2):
                hs = slice(h * NT, (h + 1) * NT)
                nc.tensor.matmul(gpsum[h], wg(k), xt(k, hs),
                                 start=(k == 0), stop=(k == KC - 1))

        def early_block(ks):
            for k in ks:
                for m in range(M_EARLY):
                    ms = slice(m * P, (m + 1) * P)
                    nc.tensor.matmul(early_pe[m][:], xt(k, ms), we(0, k, 0),
                                     start=(k == 0), stop=(k == KC - 1))

        early_block([0, 1])

        # softmax numerator: exp(logits + bg), no max-subtraction (logits
        # are bounded, |logit| < ~3 for this input distribution)
        gT_exp = gp.tile([E, BS], BF16, name="gT_exp")
        for h in range(2):
            hs = slice(h * NT, (h + 1) * NT)
            nc.scalar.activation(gT_exp[:, hs], gpsum[h],
                                 mybir.ActivationFunctionType.Exp,
                                 bias=bg_sb[:], scale=1.0)

        # transposes: [8, 128] -> [128, 8], all packed into one PSUM bank
        # (same ring slot as the gate psums — waits for the ACT reads);
        # then per-m ACT copy + accum + recip + normalize
        tp = gps.tile([P, E * MC], BF16, name="tp", tag="gq")
        accs = [[accp.tile([P, NT], F32, name=f"acc{m}_{n}", tag=f"acc{m}_{n}")
                 for n in range(NCH)] for m in range(MC)]
        g_sb, rden_sb, g8_sb = [], [], []
        for m in range(MC):
            ms = slice(m * P, (m + 1) * P)
            es = slice(m * E, (m + 1) * E)
            nc.tensor.transpose(tp[:, es], gT_exp[:, ms], ident8[:])
        early_block([2, 3, 4, 5, 6, 7])
        for m in range(MC):
            # per-m: ACT copy psum->sbuf with accumulate giving the softmax
            # denominator, then recip + normalize; the expert-0/n0 seed
            # follows immediately so each psum ring slot frees early
            es = slice(m * E, (m + 1) * E)
            ge = gp.tile([P, E], F32, name=f"ge{m}", tag=f"ge{m}")
            den = small.tile([P, 1], F32, name="den", tag="den")
            nc.scalar.activation(ge[:], tp[:, es],
                                 mybir.ActivationFunctionType.Copy,
                                 accum_out=den[:])
            rden = gp.tile([P, 1], F32, name=f"rden{m}", tag=f"rden{m}")
            nc.vector.reciprocal(rden[:], den[:])
            g = gp.tile([P, E], F32, name=f"g{m}", tag=f"g{m}")
            nc.vector.tensor_scalar_mul(g[:], ge[:], rden[:])
            g8 = gp.tile([P, N_F8], F32, name=f"g8{m}", tag=f"g8{m}")
            nc.vector.tensor_scalar_mul(g8[:], g[:, E - N_F8:E], ISCALE)
            g_sb.append(g)
            rden_sb.append(rden)
            g8_sb.append(g8)
            if m < M_EARLY:
                nc.vector.tensor_scalar_mul(accs[m][0][:], early_pe[m][:],
                                            g[:, 0:1])

        def expert_group(e, m, ns):
            """Emit matmuls for expert e, m-tile m, n-chunks ns (paired
            k-inner for stationary locality); returns psums."""
            ms = slice(m * P, (m + 1) * P)
            pes = {}
            for n in ns:
                pes[n] = eps.tile([P, NT], F32, name=f"pe{n}", tag="pe")
            for k in range(KC):
                for n in ns:
                    nc.tensor.matmul(pes[n][:], xt(k, ms), we(e, k, n),
                                     start=(k == 0), stop=(k == KC - 1))
            return pes

        def combine(e, m, n, pe):
            if e == 0:
                nc.vector.tensor_scalar_mul(accs[m][n][:], pe[:],
                                            g_sb[m][:, 0:1])
            else:
                nc.vector.scalar_tensor_tensor(
                    accs[m][n][:], pe[:], g_sb[m][:, e:e + 1],
                    accs[m][n][:], MULT, ADD)

        # ---- expert 0: finish n1 for early m-tiles, both n for the rest
        for m in range(MC):
            ns = [1] if m < M_EARLY else [0, 1]
            pes = expert_group(0, m, ns)
            for n in ns:
                combine(0, m, n, pes[n])

        def expert_group8(idx, m):
            """fp8 DoubleRow matmuls for fp8-expert idx, m-tile m: each MM
            contracts a 256-row k-pair (2 fp8 weights per PE cell)."""
            ms = slice(m * P, (m + 1) * P)
            pes = {n: eps.tile([P, NT], F32, name=f"pe8{n}", tag="pe")
                   for n in range(NCH)}
            for j in range(KC // 2):
                lhs = (xt8_all[:, j * 2 * BS:(j + 1) * 2 * BS]
                       .rearrange("p (i b) -> p i b", i=2)[:, :, ms])
                blk = (we8_all[idx][:, j * 2 * O:(j + 1) * 2 * O]
                       .rearrange("p (i c) -> p i c", i=2))
                for n in range(NCH):
                    nc.tensor.matmul(pes[n][:], lhs,
                                     blk[:, :, n * NT:(n + 1) * NT],
                                     start=(j == 0), stop=(j == KC // 2 - 1),
                                     perf_mode=mybir.MatmulPerfMode.DoubleRow)
            return pes

        # ---- experts 1..7 (trailing N_F8 in fp8 DoubleRow) ----
        for e in range(1, E):
            for m in range(MC):
                if e >= E - N_F8:
                    pes = expert_group8(e - (E - N_F8), m)
                else:
                    pes = expert_group(e, m, [0, 1])
                if e == 1:
                    # bias term: pb = gT_exp(unnorm).T @ be; the fold
                    # multiplies by rden, completing the normalization
                    ms = slice(m * P, (m + 1) * P)
                    for n in range(NCH):
                        pb = bps.tile([P, NT], F32, name="pb", tag="pb")
                        nc.tensor.matmul(pb[:], gT_exp[:, ms],
                                         be_sb[:, n * NT:(n + 1) * NT],
                                         start=True, stop=True)
                        nc.vector.scalar_tensor_tensor(
                            accs[m][n][:], pb[:], rden_sb[m][:],
                            accs[m][n][:], MULT, ADD)
                for n in range(NCH):
                    if e >= E - N_F8:
                        nc.vector.scalar_tensor_tensor(
                            accs[m][n][:], pes[n][:],
                            g8_sb[m][:, e - (E - N_F8):e - (E - N_F8) + 1],
                            accs[m][n][:], MULT, ADD)
                    else:
                        combine(e, m, n, pes[n])
                if e == E - 1:
                    ms = slice(m * P, (m + 1) * P)
                    for n in range(NCH):
                        eng = nc.scalar if n == 0 else nc.sync
                        eng.dma_start(out[ms, n * NT:(n + 1) * NT],
                                      accs[m][n][:])


_NC_CACHE = {}


def _build():
    if "nc" in _NC_CACHE:
        return _NC_CACHE["nc"]
    nc = bacc.Bacc("TRN2", target_bir_lowering=False, debug=False,
                   num_devices=NCORES)
    xT = nc.dram_tensor("xT", [P, KC * BS], BF16, kind="ExternalInput").ap()
    We_t = nc.dram_tensor("We", [E, P, KC * O], BF16,
                          kind="ExternalInput").ap()
    Wg_t = nc.dram_tensor("Wg", [D, E], BF16, kind="ExternalInput").ap()
    bg_t = nc.dram_tensor("bg", [E, 1], F32, kind="ExternalInput").ap()
    be_t = nc.dram_tensor("be", [E, O], BF16, kind="ExternalInput").ap()
    xT8_t = nc.dram_tensor("xT8", [P, KC * BS], F8, kind="ExternalInput").ap()
    We8_t = nc.dram_tensor("We8", [N_F8, P, KC * O], F8,
                           kind="ExternalInput").ap()
    out = nc.dram_tensor("out", [BS, O], F32, kind="ExternalOutput").ap()
    with tile.TileContext(nc) as tc:
        _emit(nc, tc, xT, We_t, Wg_t, bg_t, be_t, xT8_t, We8_t, out)
    nc.compile()
    _NC_CACHE["nc"] = nc
    return nc


def _in_maps(x, Wg, bg, We, be):
    bf = ml_dtypes.bfloat16
    f8 = ml_dtypes.float8_e4m3
    x = np.asarray(x, dtype=np.float32)
    We32 = np.asarray(We, dtype=np.float32)
    # We re-laid out so that for each k-chunk the (n0, n1) slices are
    # contiguous: We_r[e, p, (k*2+n)*512 + j] = We[e, k*128+p, n*512+j]
    We_bf = np.ascontiguousarray(
        We32.astype(bf)
        .reshape(E, KC, P, NCH, NT).transpose(0, 2, 1, 3, 4)
        .reshape(E, P, KC * O))
    # fp8 tail experts, pair-interleaved for DoubleRow:
    # We8_r[i, p, j*2*O + t*O + n*NT + o] = We[E-N_F8+i, (2j+t)*128+p, n*NT+o]
    We8_src = (We32[E - N_F8:].astype(bf).astype(np.float32) * WS).astype(f8)
    We8_r = np.ascontiguousarray(
        We8_src.reshape(N_F8, KC // 2, 2, P, NCH, NT)
        .transpose(0, 3, 1, 2, 4, 5).reshape(N_F8, P, KC * O))
    Wg_bf = np.asarray(Wg, dtype=np.float32).astype(bf)
    be_bf = np.asarray(be, dtype=np.float32).astype(bf)
    bg32 = np.asarray(bg, dtype=np.float32).reshape(E, 1)
    maps = []
    for c in range(NCORES):
        # xT_r[p, k*BS + b] = x[c*BS + b, k*P + p]
        xs = x[c * BS:(c + 1) * BS].astype(bf)        # [BS, D]
        xT = np.ascontiguousarray(
            xs.reshape(BS, KC, P).transpose(2, 1, 0).reshape(P, KC * BS))
        # xT8_r[p, j*2*BS + t*BS + b] = fp8(XS * x[c*BS+b, (2j+t)*128+p])
        xs8 = (xs.astype(np.float32) * XS).astype(f8)
        xT8 = np.ascontiguousarray(
            xs8.reshape(BS, KC // 2, 2, P).transpose(3, 1, 2, 0)
            .reshape(P, KC * BS))
        maps.append({"xT": xT, "We": We_bf, "Wg": Wg_bf,
                     "bg": bg32, "be": be_bf, "xT8": xT8, "We8": We8_r})
    return maps


def run(x, Wg, bg, We, be, **spmd_kwargs):
    nc = _build()
    maps = _in_maps(x, Wg, bg, We, be)
    res = run_bass_kernel_spmd(nc, maps, core_ids=list(range(NCORES)),
                               **spmd_kwargs)
    out = np.concatenate([res.results[c]["out"] for c in range(NCORES)],
                         axis=0)
    return out, res


def kernel(x, Wg, bg, We, be):
    out, _ = run(x, Wg, bg, We, be)
    return out


# revision 23
# speedup vs baseline: 1.2468x; 1.0370x over previous
"""MoE routing kernel for Trainium2 (Bass/Tile), 8-core data-parallel.

Problem: out = einsum('be,beo->bo', softmax(x@Wg+bg, axis=1),
                      einsum('bd,edo->beo', x, We) + be)
with B=8192, D=1024, O=1024, E=8 (all experts dense, softmax-weighted).

Strategy: shard the batch across 8 NeuronCores (1024 rows each). Each core:
  - gates computed TRANSPOSED: stationary Wg chunk [128,8] (M=8), moving xT
    halves -> psum gT [8, 512] x2 packed in ONE bank (col-groups 0/32);
    16 N=512 matmuls instead of 64 N=8 ones, k-ordered so they consume xT
    chunks as the DMAs land,
  - softmax: ACT Exp with per-partition bias bg[8,1] -> gT_exp [8,1024] bf16;
    thin PE transposes ([8,128] -> [128,8], ~110ns) + DVE tree-sum + recip
    give per-m normalized gates g_sb [128,8] and rden [128,1],
  - early phase: expert-0 n0 psums for m0..6 accumulate k-ordered (5 eps +
    2 bps banks) interleaved with the gate matmuls, so real PE work starts
    as soon as xT chunk0 + We[0] chunk0 arrive (~9.5us) instead of waiting
    for all of xT,
  - main loop e-outer / m-mid / k-inner with n0/n1 paired per (m,k) for
    stationary reuse; combine per (m,n) via fused DVE
    scalar_tensor_tensor acc = psum_e*g[:,e] + acc,
  - bias term: pb = gT_exp(unnormalized).T @ be per (m,n); folded with
    acc = pb*rden + acc (normalization by rden happens in the fold),
  - output DMAs alternate scalar/sync queues, issued per (m,n) right after
    the final fold.
Inputs are cast to bf16 host-side; x pre-transposed to [P, KC*BS]; We
re-laid out to [E, P, (k,n)-major] so per-chunk DMAs are contiguous.
"""
from contextlib import ExitStack

import numpy as np
import ml_dtypes

import concourse.tile as tile
import concourse.mybir as mybir
from concourse import bacc
from concourse.bass_utils import run_bass_kernel_spmd
from concourse.masks import make_identity

B, D, O, E = 8192, 1024, 1024, 8
NCORES = 8
BS = B // NCORES          # batch rows per core
P = 128                   # partition dim
NT = 512                  # matmul moving free-dim / PSUM bank width (fp32)
KC = D // P               # contraction chunks (8)
MC = BS // P              # batch-row chunks per core (8)
NCH = O // NT             # output column chunks (2)

F32 = mybir.dt.float32
BF16 = mybir.dt.bfloat16
F8 = mybir.dt.float8e4
MULT = mybir.AluOpType.mult
ADD = mybir.AluOpType.add

N_WARM = 24               # PE warm-up matmuls (cover preamble+first DMA wait)
M_EARLY = 7               # m-tiles of expert-0/n0 accumulated k-ordered early
N_F8 = 2                  # trailing experts computed in fp8 DoubleRow (2x PE)
XS, WS = 16.0, 2048.0     # fp8 scales for x and We (keep values < 240)
ISCALE = 1.0 / (XS * WS)  # folded into the combine gates for fp8 experts


def _emit(nc, tc, xT, We, Wg, bg, be, xT8, We8, out):
    ctx = ExitStack()
    with ctx:
        const = ctx.enter_context(tc.tile_pool(name="const", bufs=1))
        xp = ctx.enter_context(tc.tile_pool(name="xp", bufs=1))
        wp = ctx.enter_context(tc.tile_pool(name="wp", bufs=1))
        gp = ctx.enter_context(tc.tile_pool(name="gp", bufs=1))
        accp = ctx.enter_context(tc.tile_pool(name="accp", bufs=1))
        small = ctx.enter_context(tc.tile_pool(name="small", bufs=2))
        gps = ctx.enter_context(tc.tile_pool(name="gps", bufs=1, space="PSUM"))
        bps = ctx.enter_context(tc.tile_pool(name="bps", bufs=2, space="PSUM"))
        eps = ctx.enter_context(tc.tile_pool(name="eps", bufs=5, space="PSUM"))

        # ---- DMA issue plan ----
        # Per-core DMA bandwidth is a SHARED ~400 B/ns pool across all
        # queues, so the critical head data (xT + We[0], 4MB) must not
        # compete with the 14MB weight bulk: the bulk goes on the sync
        # queue BEHIND its critical items. Engines block IN ORDER on
        # their DMA queue's flow-control semaphores (4 outstanding), so
        # scalar (which must run the ACT softmax at ~16us) issues only 7:
        #   scalar: bg, wg, xt k0-1, we0 k0-1, xt k4-5, we0 k4-5, be
        #   sync:   xt k2-3, we0 k2-3, xt k6-7, we0 k6-7, then we1..7
        # xT / We[0] move as [128, 4KB] k-pair lines (2KB-line DMAs are
        # packet-rate-bound and crawl at ~250 B/ns aggregate).
        bg_sb = const.tile([E, 1], F32, name="bg_sb")
        nc.scalar.dma_start(bg_sb[:], bg)
        wg_all = const.tile([P, KC * E], BF16, name="wg_all")
        nc.scalar.dma_start(
            wg_all[:].rearrange("p (k e) -> p k e", k=KC),
            Wg.rearrange("(k p) e -> p k e", p=P))

        xt_all = xp.tile([P, KC * BS], BF16, name="xt_all")
        we_all = [wp.tile([P, KC * O], BF16, name=f"we{e}", tag=f"we{e}")
                  for e in range(E - N_F8)]
        xt8_all = xp.tile([P, KC * BS], F8, name="xt8_all")
        we8_all = [wp.tile([P, KC * O], F8, name=f"we8_{i}", tag=f"we8_{i}")
                   for i in range(N_F8)]

        def xt(k, ms):
            return xt_all[:, k * BS + ms.start:k * BS + ms.stop]

        def wg(k):
            return wg_all[:, k * E:(k + 1) * E]

        def we(e, k, n):
            c = (k * NCH + n) * NT
            return we_all[e][:, c:c + NT]

        def dma_xt2(eng, k):          # chunks k, k+1
            eng.dma_start(xt_all[:, k * BS:(k + 2) * BS],
                          xT[:, k * BS:(k + 2) * BS])

        def dma_we2(eng, e, k):       # chunks k, k+1 (both n-halves)
            c = k * NCH * NT
            eng.dma_start(we_all[e][:, c:c + 2 * NCH * NT],
                          We[e, :, c:c + 2 * NCH * NT])

        dma_xt2(nc.scalar, 0)
        dma_xt2(nc.sync, 2)
        dma_we2(nc.scalar, 0, 0)
        dma_we2(nc.sync, 0, 2)
        dma_xt2(nc.scalar, 4)
        dma_xt2(nc.sync, 6)
        dma_we2(nc.scalar, 0, 4)
        dma_we2(nc.sync, 0, 6)

        ident8 = const.tile([E, E], BF16, name="ident8")
        make_identity(nc, ident8[:])

        be_sb = const.tile([E, O], BF16, name="be_sb")
        nc.scalar.dma_start(be_sb[:], be)

        # bulk expert weights: all on sync, behind its critical items;
        # the fp8 tail experts + fp8 x copy load last (needed latest)
        HALF = KC * O // 2
        for e in range(1, E - N_F8):
            for h in range(2):
                nc.sync.dma_start(we_all[e][:, h * HALF:(h + 1) * HALF],
                                  We[e, :, h * HALF:(h + 1) * HALF])
        nc.sync.dma_start(xt8_all[:], xT8)
        for i in range(N_F8):
            nc.sync.dma_start(we8_all[i][:], We8[i])

        # ---- PE warm-up ----
        # HAM keeps the PE clock-gated at 1.2 GHz until ~3.4us of sustained
        # matmul activity; the first ~8us is also DMA-wait (framework
        # preamble + first chunk transfers). Burn warm-up matmuls there.
        warm_sb = const.tile([P, NT], BF16, name="warm_sb")
        nc.vector.memset(warm_sb[:], 0.0)

        def warmup(n):
            for _ in range(n):
                pwu = bps.tile([P, NT], F32, name="pwu", tag="pb")
                nc.tensor.matmul(pwu[:], warm_sb[:, :P], warm_sb[:],
                                 start=True, stop=True)

        warmup(N_WARM)

        # ---- chunk-paced phase: gates (transposed) + expert-0/n0 ----
        # gate psums: two [8, 512] tiles packed into one PSUM bank at
        # base partitions 0 and 32 (col-groups run concurrently on PE)
        g4 = gps.tile([40, NT], F32, name="g4", tag="gq")
        gpsum = [g4[0:E, :], g4[32:32 + E, :]]

        # early expert-0 n0 psums: m0..4 from the shared eps ring (bufs=5),
        # m5..6 from the bps ring (bufs=2) — ring FIFO order matches the
        # seed order below, so the main loop's allocations naturally wait
        # on the seeds
        early_pe = []
        for m in range(M_EARLY):
            pool, tag = (eps, "pe") if m < 5 else (bps, "pb")
            t = pool.tile([P, NT], F32, name=f"pe_e{m}", tag=tag)
            early_pe.append(t)

        # gates first (paced only by the xT DMAs) so the softmax pipeline
        # starts as early as possible
        for k in range(KC):
            for h in range(2):
                hs = slice(h * NT, (h + 1) * NT)
                nc.tensor.matmul(gpsum[h], wg(k), xt(k, hs),
                                 start=(k == 0), stop=(k == KC - 1))

        def early_block(ks):
            for k in ks:
                for m in range(M_EARLY):
                    ms = slice(m * P, (m + 1) * P)
                    nc.tensor.matmul(early_pe[m][:], xt(k, ms), we(0, k, 0),
                                     start=(k == 0), stop=(k == KC - 1))

        early_block([0, 1])

        # softmax numerator: exp(logits + bg), no max-subtraction (logits
        # are bounded, |logit| < ~3 for this input distribution)
        gT_exp = gp.tile([E, BS], BF16, name="gT_exp")
        for h in range(2):
            hs = slice(h * NT, (h + 1) * NT)
            nc.scalar.activation(gT_exp[:, hs], gpsum[h],
                                 mybir.ActivationFunctionType.Exp,
                                 bias=bg_sb[:], scale=1.0)

        # transposes: [8, 128] -> [128, 8], all packed into one PSUM bank
        # (same ring slot as the gate psums — waits for the ACT reads);
        # then per-m ACT copy + accum + recip + normalize
        tp = gps.tile([P, E * MC], BF16, name="tp", tag="gq")
        accs = [[accp.tile([P, NT], F32, name=f"acc{m}_{n}", tag=f"acc{m}_{n}")
                 for n in range(NCH)] for m in range(MC)]
        g_sb, rden_sb, g8_sb = [], [], []
        for m in range(MC):
            ms = slice(m * P, (m + 1) * P)
            es = slice(m * E, (m + 1) * E)
            nc.tensor.transpose(tp[:, es], gT_exp[:, ms], ident8[:])
        early_block([2, 3, 4, 5, 6, 7])
        for m in range(MC):
            # per-m: ACT copy psum->sbuf with accumulate giving the softmax
            # denominator, then recip + normalize; the expert-0/n0 seed
            # follows immediately so each psum ring slot frees early
            es = slice(m * E, (m + 1) * E)
            ge = gp.tile([P, E], F32, name=f"ge{m}", tag=f"ge{m}")
            den = small.tile([P, 1], F32, name="den", tag="den")
            nc.scalar.activation(ge[:], tp[:, es],
                                 mybir.ActivationFunctionType.Copy,
                                 accum_out=den[:])
            rden = gp.tile([P, 1], F32, name=f"rden{m}", tag=f"rden{m}")
            nc.vector.reciprocal(rden[:], den[:])
            g = gp.tile([P, E], F32, name=f"g{m}", tag=f"g{m}")
            nc.vector.tensor_scalar_mul(g[:], ge[:], rden[:])
            g8 = gp.tile([P, N_F8], F32, name=f"g8{m}", tag=f"g8{m}")
            nc.vector.tensor_scalar_mul(g8[:], g[:, E - N_F8:E], ISCALE)
            g_sb.append(g)
            rden_sb.append(rden)
            g8_sb.append(g8)
            if m < M_EARLY:
                nc.vector.tensor_scalar_mul(accs[m][0][:], early_pe[m][:],
                                            g[:, 0:1])

        def expert_group(e, m, ns):
            """Emit matmuls for expert e, m-tile m, n-chunks ns (paired
            k-inner for stationary locality); returns psums."""
            ms = slice(m * P, (m + 1) * P)
            pes = {}
            for n in ns:
                pes[n] = eps.tile([P, NT], F32, name=f"pe{n}", tag="pe")
            for k in range(KC):
                for n in ns:
                    nc.tensor.matmul(pes[n][:], xt(k, ms), we(e, k, n),
                                     start=(k == 0), stop=(k == KC - 1))
            return pes

        def combine(e, m, n, pe):
            if e == 0:
                nc.vector.tensor_scalar_mul(accs[m][n][:], pe[:],
                                            g_sb[m][:, 0:1])
            else:
                nc.vector.scalar_tensor_tensor(
                    accs[m][n][:], pe[:], g_sb[m][:, e:e + 1],
                    accs[m][n][:], MULT, ADD)

        # ---- expert 0: finish n1 for early m-tiles, both n for the rest
        for m in range(MC):
            ns = [1] if m < M_EARLY else [0, 1]
            pes = expert_group(0, m, ns)
            for n in ns:
                combine(0, m, n, pes[n])

        def expert_group8(idx, m):
            """fp8 DoubleRow matmuls for fp8-expert idx, m-tile m: each MM
            contracts a 256-row k-pair (2 fp8 weights per PE cell)."""
            ms = slice(m * P, (m + 1) * P)
            pes = {n: eps.tile([P, NT], F32, name=f"pe8{n}", tag="pe")
                   for n in range(NCH)}
            for j in range(KC // 2):
                lhs = (xt8_all[:, j * 2 * BS:(j + 1) * 2 * BS]
                       .rearrange("p (i b) -> p i b", i=2)[:, :, ms])
                blk = (we8_all[idx][:, j * 2 * O:(j + 1) * 2 * O]
                       .rearrange("p (i c) -> p i c", i=2))
                for n in range(NCH):
                    nc.tensor.matmul(pes[n][:], lhs,
                                     blk[:, :, n * NT:(n + 1) * NT],
                                     start=(j == 0), stop=(j == KC // 2 - 1),
                                     perf_mode=mybir.MatmulPerfMode.DoubleRow)
            return pes

        # ---- experts 1..7 (trailing N_F8 in fp8 DoubleRow) ----
        for e in range(1, E):
            for m in range(MC):
                if e >= E - N_F8:
                    pes = expert_group8(e - (E - N_F8), m)
                else:
                    pes = expert_group(e, m, [0, 1])
                if e == 1:
                    # bias term: pb = gT_exp(unnorm).T @ be; the fold
                    # multiplies by rden, completing the normalization
                    ms = slice(m * P, (m + 1) * P)
                    for n in range(NCH):
                        pb = bps.tile([P, NT], F32, name="pb", tag="pb")
                        nc.tensor.matmul(pb[:], gT_exp[:, ms],
                                         be_sb[:, n * NT:(n + 1) * NT],
                                         start=True, stop=True)
                        nc.vector.scalar_tensor_tensor(
                            accs[m][n][:], pb[:], rden_sb[m][:],
                            accs[m][n][:], MULT, ADD)
                for n in range(NCH):
                    if e >= E - N_F8:
                        nc.vector.scalar_tensor_tensor(
                            accs[m][n][:], pes[n][:],
                            g8_sb[m][:, e - (E - N_F8):e - (E - N_F8) + 1],
                            accs[m][n][:], MULT, ADD)
                    else:
                        combine(e, m, n, pes[n])
                if e == E - 1:
                    ms = slice(m * P, (m + 1) * P)
                    for n in range(NCH):
                        eng = nc.scalar if n == 0 else nc.sync
                        eng.dma_start(out[ms, n * NT:(n + 1) * NT],
                                      accs[m][n][:])


_NC_CACHE = {}


def _build():
    if "nc" in _NC_CACHE:
        return _NC_CACHE["nc"]
    nc = bacc.Bacc("TRN2", target_bir_lowering=False, debug=False,
                   num_devices=NCORES)
    xT = nc.dram_tensor("xT", [P, KC * BS], BF16, kind="ExternalInput").ap()
    We_t = nc.dram_tensor("We", [E - N_F8, P, KC * O], BF16,
                          kind="ExternalInput").ap()
    Wg_t = nc.dram_tensor("Wg", [D, E], BF16, kind="ExternalInput").ap()
    bg_t = nc.dram_tensor("bg", [E, 1], F32, kind="ExternalInput").ap()
    be_t = nc.dram_tensor("be", [E, O], BF16, kind="ExternalInput").ap()
    xT8_t = nc.dram_tensor("xT8", [P, KC * BS], F8, kind="ExternalInput").ap()
    We8_t = nc.dram_tensor("We8", [N_F8, P, KC * O], F8,
                           kind="ExternalInput").ap()
    out = nc.dram_tensor("out", [BS, O], F32, kind="ExternalOutput").ap()
    with tile.TileContext(nc) as tc:
        _emit(nc, tc, xT, We_t, Wg_t, bg_t, be_t, xT8_t, We8_t, out)
    nc.compile()
    _NC_CACHE["nc"] = nc
    return nc


def _in_maps(x, Wg, bg, We, be):
    bf = ml_dtypes.bfloat16
    f8 = ml_dtypes.float8_e4m3
    x = np.asarray(x, dtype=np.float32)
    We32 = np.asarray(We, dtype=np.float32)
    # We re-laid out so that for each k-chunk the (n0, n1) slices are
    # contiguous: We_r[e, p, (k*2+n)*512 + j] = We[e, k*128+p, n*512+j]
    We_bf = np.ascontiguousarray(
        We32.astype(bf)
        .reshape(E, KC, P, NCH, NT).transpose(0, 2, 1, 3, 4)
        .reshape(E, P, KC * O))
    # fp8 tail experts, pair-interleaved for DoubleRow:
    # We8_r[i, p, j*2*O + t*O + n*NT + o] = We[E-N_F8+i, (2j+t)*128+p, n*NT+o]
    We8_src = (We32[E - N_F8:].astype(bf).astype(np.float32) * WS).astype(f8)
    We8_r = np.ascontiguousarray(
        We8_src.reshape(N_F8, KC // 2, 2, P, NCH, NT)
        .transpose(0, 3, 1, 2, 4, 5).reshape(N_F8, P, KC * O))
    Wg_bf = np.asarray(Wg, dtype=np.float32).astype(bf)
    be_bf = np.asarray(be, dtype=np.float32).astype(bf)
    bg32 = np.asarray(bg, dtype=np.float32).reshape(E, 1)
    maps = []
    NB = E - N_F8
    for c in range(NCORES):
        # Rotate the bf16 experts per core (keeping the fp8 pair last):
        # mathematically invariant — Wg columns / bg / be rows / We expert
        # order permute together — but each core's early critical DMA then
        # reads a DIFFERENT 2MB weight block, decorrelating HBM contention
        # across the 8 cores.
        perm = [(c + e) % NB for e in range(NB)] + list(range(NB, E))
        # xT_r[p, k*BS + b] = x[c*BS + b, k*P + p]
        xs = x[c * BS:(c + 1) * BS].astype(bf)        # [BS, D]
        xT = np.ascontiguousarray(
            xs.reshape(BS, KC, P).transpose(2, 1, 0).reshape(P, KC * BS))
        # xT8_r[p, j*2*BS + t*BS + b] = fp8(XS * x[c*BS+b, (2j+t)*128+p])
        xs8 = (xs.astype(np.float32) * XS).astype(f8)
        xT8 = np.ascontiguousarray(
            xs8.reshape(BS, KC // 2, 2, P).transpose(3, 1, 2, 0)
            .reshape(P, KC * BS))
        maps.append({"xT": xT, "We": np.ascontiguousarray(We_bf[perm[:NB]]),
                     "Wg": np.ascontiguousarray(Wg_bf[:, perm]),
                     "bg": bg32[perm], "be": np.ascontiguousarray(be_bf[perm]),
                     "xT8": xT8, "We8": We8_r})
    return maps


def run(x, Wg, bg, We, be, **spmd_kwargs):
    nc = _build()
    maps = _in_maps(x, Wg, bg, We, be)
    res = run_bass_kernel_spmd(nc, maps, core_ids=list(range(NCORES)),
                               **spmd_kwargs)
    out = np.concatenate([res.results[c]["out"] for c in range(NCORES)],
                         axis=0)
    return out, res


def kernel(x, Wg, bg, We, be):
    out, _ = run(x, Wg, bg, We, be)
    return out
